# revision 1
# baseline (speedup 1.0000x reference)
"""GroupSort (k=4) Trainium2 Bass kernel.

x: (16384, 4096) f32. Sort each contiguous group of 4 along the last dim.
Sharding: batch-parallel across 8 NeuronCores (2048 rows/core), no comms.

Per core: the 2048x4096 shard is 16 tiles of [128 partitions, 4096 free].
A 5-comparator sorting network sorts every contiguous group of 4. DVE ops
with any stride-4 operand run at ~0.59 elem/cycle (measured), so the
network is restructured: pair stages read stride-2 even/odd views and
write contiguous temps, and the four unavoidable stride-4 interleave
writes into the output tile are done by the otherwise-idle Scalar engine
as copies. Raw Bass program (Tile's semaphore pass emits multi-wait DMA
instructions, which the single-wait DIRECT2D ISA struct rejects; walrus
also rejects TensorTensor on Pool in this toolchain):

  SP ring:  loads (HWDGE), double-buffered input
  ACT ring: 4 interleave copies per tile + stores (HWDGE)
  DVE:      8 min/max ops per tile into contiguous temps

Roofline: 64 MiB HBM traffic/core at ~358 GB/s = ~187 us.
"""

import numpy as np

B, D, K = 16384, 4096, 4
NCORES = 8
RPC = B // NCORES  # rows per core
N = RPC * D  # flat elements per core
P = 128  # SBUF partitions
F = 4096  # free-dim elements per tile
G = F // K  # groups per partition per tile
G2 = F // 2
NTILES = N // (P * F)  # 16
NBUF = 3

_cache = {}


def _build():
    import concourse.bass as bass
    import concourse.mybir as mybir

    fp32 = mybir.dt.float32
    mn = mybir.AluOpType.min
    mx = mybir.AluOpType.max

    nc = bass.Bass()
    x = nc.dram_tensor("x", [N], fp32, kind="ExternalInput")
    y = nc.dram_tensor("y", [N], fp32, kind="ExternalOutput")
    x_t = x[:].rearrange("(n p f) -> n p f", p=P, f=F)
    y_t = y[:].rearrange("(n p f) -> n p f", p=P, f=F)

    with (
        nc.sbuf_tensor([P, NBUF * F], fp32) as tin,
        nc.sbuf_tensor([P, NBUF * F], fp32) as tout,
        nc.sbuf_tensor([P, F], fp32) as pairs,  # [lo01 lo23..|hi01 hi23..]
        # handoff slot layout: [q0(2G)=l0|m2, q1(2G)=m1|l3, l1(G), l2(G)]
        nc.sbuf_tensor([P, NBUF * 6 * G], fp32) as lanes,
        nc.semaphore("dma_in") as dma_in,
        nc.semaphore("dma_out") as dma_out,
        nc.semaphore("ve") as ve,
        nc.semaphore("ac") as ac,
        nc.Block() as block,
    ):

        @block.sync
        def _(sync):
            for i in range(NTILES):
                if i > 0:
                    # order completions (also satisfies the sim's sem rule)
                    sync.wait_ge(dma_in, 16 * i)
                if i >= NBUF:
                    # in-slot reuse: stage-1 of tile i-NBUF consumed it
                    sync.wait_ge(ve, 2 * (i - NBUF) + 1)
                sync.dma_start(
                    tin[:, i % NBUF * F : (i % NBUF + 1) * F], x_t[i]
                ).then_inc(dma_in, 16)

        @block.vector
        def _(vector):
            for i in range(NTILES):
                s = i % NBUF
                vi = tin[:, s * F : (s + 1) * F].rearrange(
                    "p (g k) -> p g k", k=2
                )
                ev, od = vi[:, :, 0], vi[:, :, 1]  # stride-2 views
                vp = pairs[:].rearrange("p (g k) -> p g k", k=2)
                base = 6 * s * G
                q0 = lanes[:, base : base + 2 * G]  # [l0 | m2]
                q1 = lanes[:, base + 2 * G : base + 4 * G]  # [m1 | l3]
                l1 = lanes[:, base + 4 * G : base + 5 * G]
                l2 = lanes[:, base + 5 * G : base + 6 * G]

                vector.wait_ge(dma_in, 16 * (i + 1))
                # stage 1: two comparators per op — lo half then hi half of
                # the pairs buffer; stride-2 reads, unit writes
                vector.tensor_tensor(pairs[:, :G2], ev, od, mn)
                vector.tensor_tensor(pairs[:, G2:], ev, od, mx)
                # inc: tells the SP ring the input slot is free
                vector.drain().then_inc(ve, 1)
                if i >= NBUF:
                    # handoff-slot reuse: ACT copies of tile i-NBUF done
                    vector.wait_ge(ac, i - NBUF + 1)
                # stage 2: again two comparators per op over the full pairs
                # buffer: min -> [min(lo01,lo23)|min(hi01,hi23)] = [l0|m2],
                # max -> [max(lo01,lo23)|max(hi01,hi23)] = [m1|l3]
                vector.tensor_tensor(q0, vp[:, :, 0], vp[:, :, 1], mn)
                vector.tensor_tensor(q1, vp[:, :, 0], vp[:, :, 1], mx)
                vector.drain()
                # stage 3: fully unit; m1 = q1[:G], m2 = q0[G:]
                vector.tensor_tensor(l1, q1[:, :G], q0[:, G:], mn)
                vector.tensor_tensor(l2, q1[:, :G], q0[:, G:], mx)
                # commit before the ACT ring interleaves this tile
                vector.drain().then_inc(ve, 1)

        @block.scalar
        def _(scalar):
            for i in range(NTILES):
                s = i % NBUF
                vo = tout[:, s * F : (s + 1) * F].rearrange(
                    "p (g k) -> p g k", k=K
                )
                base = 6 * s * G
                ln = [
                    lanes[:, base : base + G],  # l0 = q0[:G]
                    lanes[:, base + 4 * G : base + 5 * G],  # l1
                    lanes[:, base + 5 * G : base + 6 * G],  # l2
                    lanes[:, base + 3 * G : base + 4 * G],  # l3 = q1[G:]
                ]
                scalar.wait_ge(ve, 2 * i + 2)
                if i >= NBUF:
                    # out-slot reuse: store of tile i-NBUF has drained
                    scalar.wait_ge(dma_out, 16 * (i - NBUF + 1))
                for j in range(4):
                    scalar.copy(vo[:, :, j], ln[j])
                # commit copies, free the handoff slot for DVE
                scalar.drain().then_inc(ac, 1)
                if i > 0:
                    scalar.wait_ge(dma_out, 16 * i)
                scalar.dma_start(
                    y_t[i], tout[:, s * F : (s + 1) * F]
                ).then_inc(dma_out, 16)

    return nc


def _run(x_np, trace=False, trace_kwargs=None):
    from concourse.bass_utils import run_bass_kernel_spmd

    if "nc" not in _cache:
        _cache["nc"] = _build()
    nc = _cache["nc"]

    shards = np.split(np.ascontiguousarray(x_np, dtype=np.float32), NCORES, axis=0)
    in_maps = [{"x": s.reshape(-1)} for s in shards]
    res = run_bass_kernel_spmd(
        nc,
        in_maps,
        list(range(NCORES)),
        trace=trace,
        **(trace_kwargs or {}),
    )
    out = np.concatenate([r["y"].reshape(RPC, D) for r in res.results], axis=0)
    return out, res


def kernel(x, k):
    assert int(k) == K, f"kernel hardcodes k={K}, got {k}"
    out, _ = _run(np.asarray(x))
    return out

